# revision 1
# baseline (speedup 1.0000x reference)
"""GQA attention kernel (B=1, S=2048, D=4096, 32 Q heads / 8 KV heads, RoPE,
causal) for 8 Trainium2 NeuronCores.

Sharding: tensor-parallel over heads. Core c owns Q heads 4c..4c+3 and KV head
c (whole GQA group), computes its context slice and a partial o-projection
(rows 512c..512c+511 of Wo); the host sums the 8 partial outputs.

Layout: activations are kept feature-on-partition ("transposed"): hsT [D, S],
qT/kT/vT [128, S]. All matmuls run in float32r (fp32 with 11-bit mantissa,
full PE speed) on host-pre-rounded data; PSUM accumulation is fp32.

RoPE: rotate_half is a signed permutation R, applied as a [128,128] matmul
(lhsT = R^T), then q_rope = q*cos + (Rq)*sin elementwise on DVE.

Softmax: no max-subtraction (logits are O(10) here, exp is safe in fp32);
exp on ScalarE; row-sums via ones-vector matmul on the PE (M=1); the
1/rowsum normalization is broadcast across partitions on the GPSIMD.
"""
import numpy as np
import ml_dtypes
from contextlib import ExitStack

try:  # reuse compiled executables across processes when possible
    import jax
    jax.config.update("jax_compilation_cache_dir", "/tmp/jax_comp_cache")
    jax.config.update("jax_persistent_cache_min_entry_size_bytes", -1)
    jax.config.update("jax_persistent_cache_min_compile_time_secs", 1.0)
except Exception:
    pass

import concourse.bacc as bacc
import concourse.tile as tile
import concourse.mybir as mybir
from concourse.bass_utils import run_bass_kernel_spmd

F32 = mybir.dt.float32
F32R = mybir.dt.float32r
BF16 = mybir.dt.bfloat16

S = 2048            # sequence length
D = 4096            # hidden dim
HD = 128            # head dim
NCORES = 8
QH = 4              # q heads per core
KT = D // 128       # 32 contraction tiles for the projections
NCHUNK = S // 512   # 4 sequence chunks of 512
NJT = S // 128      # 16 key tiles of 128
INV_SQRT_D = float(1.0 / np.sqrt(np.float32(HD)))
NEG_INF = -3.4e38
ROPE_BASE = 10000.0


def round_fp32r(x: np.ndarray) -> np.ndarray:
    """Round fp32 to fp32r (11 mantissa bits, RNE); low 12 bits zeroed."""
    u = np.ascontiguousarray(x, dtype=np.float32).view(np.uint32)
    keep = (u >> 12) & np.uint32(1)
    u = u + np.uint32(0x7FF) + keep
    u = u & np.uint32(0xFFFFF000)
    return u.view(np.float32)


def _build_nc():
    nc = bacc.Bacc(None)

    hst_d = nc.dram_tensor("hst", [D, S], F32R, kind="ExternalInput")
    wq_d = nc.dram_tensor("wq", [D, QH * HD], F32R, kind="ExternalInput")
    wk_d = nc.dram_tensor("wk", [D, HD], F32R, kind="ExternalInput")
    wv_d = nc.dram_tensor("wv", [D, HD], F32R, kind="ExternalInput")
    wo_d = nc.dram_tensor("wo", [QH * HD, D], F32R, kind="ExternalInput")
    cos_d = nc.dram_tensor("cost", [HD, S], F32, kind="ExternalInput")
    sin_d = nc.dram_tensor("sint", [HD, S], F32, kind="ExternalInput")
    mask_d = nc.dram_tensor("maskt", [128, 4, 512], BF16, kind="ExternalInput")
    rt_d = nc.dram_tensor("rt", [128, 128], F32R, kind="ExternalInput")
    ident_d = nc.dram_tensor("ident", [128, 128], F32R, kind="ExternalInput")
    ones_d = nc.dram_tensor("ones", [128, 128], F32R, kind="ExternalInput")
    out_d = nc.dram_tensor("out", [S, D], F32, kind="ExternalOutput")

    with tile.TileContext(nc) as tc, ExitStack() as ctx:
        wpool = ctx.enter_context(tc.tile_pool(name="wpool", bufs=1))
        cpool = ctx.enter_context(tc.tile_pool(name="cpool", bufs=1))
        big = ctx.enter_context(tc.tile_pool(name="bigacts", bufs=1))
        hsp = ctx.enter_context(tc.tile_pool(name="hsp", bufs=8))
        evac = ctx.enter_context(tc.tile_pool(name="evac", bufs=6))
        qrp = ctx.enter_context(tc.tile_pool(name="qrp", bufs=4))
        tmp = ctx.enter_context(tc.tile_pool(name="tmp", bufs=2))
        ptp = ctx.enter_context(tc.tile_pool(name="ptp", bufs=5))
        smal = ctx.enter_context(tc.tile_pool(name="smal", bufs=1))
        rbp = ctx.enter_context(tc.tile_pool(name="rbp", bufs=1))
        ctxn = ctx.enter_context(tc.tile_pool(name="ctxn", bufs=2))
        cin = ctx.enter_context(tc.tile_pool(name="cin", bufs=2))
        osb = ctx.enter_context(tc.tile_pool(name="osb", bufs=3))
        psum = ctx.enter_context(tc.tile_pool(name="psum", bufs=8, space="PSUM"))
        dram = ctx.enter_context(tc.tile_pool(name="dram", bufs=1, space="DRAM"))

        # ---- resident weights & constants ----
        # weights are loaded with one DMA per contraction tile so the first
        # matmuls only wait for their own slice, not the whole 8.4 MB
        wq_sb = wpool.tile([128, KT, QH * HD], F32R, tag="wqo")
        wq_r = wq_d[:, :].rearrange("(t p) m -> p t m", p=128)
        wk_sb = wpool.tile([128, KT, HD], F32R, tag="wk")
        wk_r = wk_d[:, :].rearrange("(t p) m -> p t m", p=128)
        wv_sb = wpool.tile([128, KT, HD], F32R, tag="wv")
        wv_r = wv_d[:, :].rearrange("(t p) m -> p t m", p=128)
        # first weight slices only — the rest are interleaved into the first
        # chunk's t-loop so the sync queue's serial descriptor-generation
        # doesn't delay the pipeline start
        nc.sync.dma_start(out=wq_sb[:, 0:1, :], in_=wq_r[:, 0:1, :])
        nc.sync.dma_start(out=wq_sb[:, 1:4, :], in_=wq_r[:, 1:4, :])
        nc.sync.dma_start(out=wk_sb[:, 0:8, :], in_=wk_r[:, 0:8, :])
        nc.sync.dma_start(out=wv_sb[:, 0:8, :], in_=wv_r[:, 0:8, :])

        cos_sb = cpool.tile([HD, S], F32, tag="cos")
        sin_sb = cpool.tile([HD, S], F32, tag="sin")
        mask_sb = cpool.tile([128, 4, 512], BF16, tag="mask")
        rt_sb = cpool.tile([128, 128], F32R, tag="rt")
        ident_sb = cpool.tile([128, 128], F32R, tag="ident")
        ones_sb = cpool.tile([128, 128], F32R, tag="ones")

        def _late_loads(t):
            # emitted inside the first chunk's t-loop, just ahead of first use
            if t % 4 == 1 and t < 29:
                g = t // 4 + 1
                nc.sync.dma_start(out=wq_sb[:, 4 * g:4 * g + 4, :],
                                  in_=wq_r[:, 4 * g:4 * g + 4, :])
            if t == 3:
                nc.sync.dma_start(out=wk_sb[:, 8:16, :], in_=wk_r[:, 8:16, :])
                nc.sync.dma_start(out=wv_sb[:, 8:16, :], in_=wv_r[:, 8:16, :])
            elif t == 10:
                nc.sync.dma_start(out=wk_sb[:, 16:32, :], in_=wk_r[:, 16:32, :])
                nc.sync.dma_start(out=wv_sb[:, 16:32, :], in_=wv_r[:, 16:32, :])
            elif t == 14:
                nc.gpsimd.dma_start(out=cos_sb[:], in_=cos_d[:, :])
                nc.gpsimd.dma_start(out=sin_sb[:], in_=sin_d[:, :])
            elif t == 18:
                nc.gpsimd.dma_start(out=rt_sb[:], in_=rt_d[:, :])
                nc.gpsimd.dma_start(out=ident_sb[:], in_=ident_d[:, :])
                nc.gpsimd.dma_start(out=ones_sb[:], in_=ones_d[:, :])
            elif t == 22:
                nc.gpsimd.dma_start(out=mask_sb[:], in_=mask_d[:, :, :])

        krope_sb = big.tile([128, S], F32R, tag="krope")   # kT after rope
        vnat_sb = big.tile([128, S], F32R, tag="vnat")     # v natural [j, d] blocks

        ctx_dram = dram.tile([QH * HD, S], F32R)           # normalized context^T

        # ---- fused per-chunk pipeline ----
        for icnk in range(NCHUNK):
            c0, c1 = icnk * 512, (icnk + 1) * 512
            # joint qkv projection for this chunk: 6 accumulators (q0..q3, k, v)
            accs = [psum.tile([128, 512], F32, tag="ps", name=f"acc{icnk}_{i}")
                    for i in range(6)]
            for t in range(KT):
                hst_t = hsp.tile([128, 512], F32R, tag="hst")
                nc.sync.dma_start(out=hst_t[:], in_=hst_d[t * 128:(t + 1) * 128, c0:c1])
                if icnk == 0:
                    _late_loads(t)
                for m in range(6):
                    if m < 4:
                        lhsT = wq_sb[:, t, m * HD:(m + 1) * HD]
                    elif m == 4:
                        lhsT = wk_sb[:, t, :]
                    else:
                        lhsT = wv_sb[:, t, :]
                    nc.tensor.matmul(accs[m][:], lhsT, hst_t[:],
                                     start=(t == 0), stop=(t == KT - 1))

            def _rope(m):
                # evacuate acc[m] and apply RoPE; returns the rotated chunk
                ch = evac.tile([128, 512], F32R, tag="evac", name=f"ch{icnk}_{m}")
                nc.vector.tensor_copy(ch[:], accs[m][:])
                rot = psum.tile([128, 512], F32, tag="ps", name=f"rot{icnk}_{m}")
                nc.tensor.matmul(rot[:], rt_sb[:], ch[:], start=True, stop=True)
                t1 = tmp.tile([128, 512], F32, tag="t1", name=f"t1_{icnk}_{m}")
                nc.vector.tensor_mul(t1[:], ch[:].bitcast(F32), cos_sb[:, c0:c1])
                t2 = tmp.tile([128, 512], F32, tag="t2", name=f"t2_{icnk}_{m}")
                nc.vector.tensor_mul(t2[:], rot[:], sin_sb[:, c0:c1])
                if m < 4:
                    dest = qrp.tile([128, 512], F32R, tag="qrp", name=f"qr{icnk}_{m}")
                    nc.vector.tensor_add(dest[:], t1[:], t2[:])
                    return dest
                nc.vector.tensor_add(krope_sb[:, c0:c1], t1[:], t2[:])
                return None

            _rope(4)
            chv = evac.tile([128, 512], F32R, tag="evac")
            nc.vector.tensor_copy(chv[:], accs[5][:])
            for tt in range(4):
                jt = icnk * 4 + tt
                vt_ps = psum.tile([128, 128], F32R, tag="ps", name=f"vt{icnk}_{tt}")
                nc.tensor.matmul(vt_ps[:], chv[:, tt * 128:(tt + 1) * 128],
                                 ident_sb[:], is_transpose=True,
                                 start=True, stop=True)
                nc.vector.tensor_copy(vnat_sb[:, jt * 128:(jt + 1) * 128], vt_ps[:])
            qrope_chunks = [_rope(m) for m in range(4)]

            # attention for the 4 heads, query chunk = icnk (keys 0..4*icnk+3)
            jt_max = icnk * 4 + 3
            for h in range(QH):
                qr = qrope_chunks[h]
                ctx_acc = psum.tile([128, 512], F32, tag="ps")
                rs_acc = psum.tile([1, 512], F32, tag="ps")
                # software-pipelined: the rowsum/av matmuls for tile jt are
                # emitted after the scores matmul of jt+1, so the (in-order)
                # PE never waits on the mask->exp chain
                pending = []
                LOOKAHEAD = 2

                def _consume(pjt, ppT, last):
                    nc.tensor.matmul(rs_acc[:], ones_sb[:, 0:1], ppT[:],
                                     start=(pjt == 0), stop=last)
                    nc.tensor.matmul(ctx_acc[:], vnat_sb[:, pjt * 128:(pjt + 1) * 128],
                                     ppT[:], start=(pjt == 0), stop=last)

                for jt in range(jt_max + 1):
                    sT = psum.tile([128, 512], F32, tag="ps")
                    nc.tensor.matmul(sT[:], krope_sb[:, jt * 128:(jt + 1) * 128],
                                     qr[:], start=True, stop=True)
                    if len(pending) >= LOOKAHEAD:
                        _consume(*pending.pop(0), False)
                    if jt >= icnk * 4:
                        r = jt - icnk * 4
                        nc.vector.tensor_add(sT[:], sT[:], mask_sb[:, r, :])
                    pT = ptp.tile([128, 512], F32R, tag="pt")
                    nc.scalar.activation(out=pT[:], in_=sT[:],
                                         func=mybir.ActivationFunctionType.Exp,
                                         scale=INV_SQRT_D)
                    pending.append((jt, pT))
                while pending:
                    _consume(*pending.pop(0), len(pending) == 0)
                # normalize: ctx * (1/rowsum); broadcast over partitions on the
                # (otherwise idle) GPSIMD so the PE never stalls on this chain
                recip = smal.tile([1, 512], F32, tag="recip")
                with nc.allow_low_precision(reason="softmax denominator reciprocal"):
                    nc.vector.reciprocal(recip[:], rs_acc[:])
                rb_sb = rbp.tile([128, 512], F32, tag="rb")
                nc.gpsimd.partition_broadcast(rb_sb[:], recip[:])
                cn = ctxn.tile([128, 512], F32R, tag="cn")
                nc.vector.tensor_mul(cn[:], ctx_acc[:], rb_sb[:])
                nc.gpsimd.dma_start(out=ctx_dram[h * HD:(h + 1) * HD, c0:c1], in_=cn[:])

        # ---- partial o-projection: out[s, e] = sum_j ctxT[j, s] * wo[j, e] ----
        wo_sb = wpool.tile([128, 4, D], F32R, tag="wqo")  # reuses wq slot
        wo_r = wo_d[:, :].rearrange("(t p) e -> p t e", p=128)
        for jt in range(4):
            nc.gpsimd.dma_start(out=wo_sb[:, jt, :], in_=wo_r[:, jt, :])
        for st in range(NJT):
            ci = cin.tile([128, 4, 128], F32R, tag="ci")
            nc.gpsimd.dma_start(
                out=ci[:],
                in_=ctx_dram[:, st * 128:(st + 1) * 128].rearrange("(t p) s -> p t s", p=128),
            )
            for ec in range(8):
                oacc = psum.tile([128, 512], F32, tag="ps")
                for jt in range(4):
                    nc.tensor.matmul(oacc[:], ci[:, jt, :],
                                     wo_sb[:, jt, ec * 512:(ec + 1) * 512],
                                     start=(jt == 0), stop=(jt == 3))
                ot = osb.tile([128, 512], F32, tag="ot")
                nc.vector.tensor_copy(ot[:], oacc[:])
                nc.scalar.dma_start(out=out_d[st * 128:(st + 1) * 128, ec * 512:(ec + 1) * 512],
                                  in_=ot[:])

    nc.finalize()
    return nc


_NC_CACHE = None


def _host_tables():
    inv_freq = 1.0 / (ROPE_BASE ** (np.arange(0, HD, 2, dtype=np.float32) / HD))
    pos = np.arange(S, dtype=np.float32)
    freqs = pos[:, None] * inv_freq[None, :].astype(np.float32)   # [S, 64]
    emb = np.concatenate([freqs, freqs], axis=1).astype(np.float32)  # [S, 128]
    cosT = np.ascontiguousarray(np.cos(emb).astype(np.float32).T)  # [128, S]
    sinT = np.ascontiguousarray(np.sin(emb).astype(np.float32).T)

    # causal mask for diagonal blocks, transposed orientation [jp, r, if]
    jp = np.arange(128)[:, None, None]
    r = np.arange(4)[None, :, None]
    iF = np.arange(512)[None, None, :]
    mask = np.where(r * 128 + jp <= iF, 0.0, NEG_INF).astype(np.float32)
    mask_bf = mask.astype(ml_dtypes.bfloat16)

    rt = np.zeros((128, 128), dtype=np.float32)
    idx = np.arange(64)
    rt[idx + 64, idx] = -1.0
    rt[idx, idx + 64] = 1.0

    ident = np.eye(128, dtype=np.float32)
    ones = np.ones((128, 128), dtype=np.float32)
    return cosT, sinT, mask_bf, rt, ident, ones


def kernel(hidden_states, Wq, Wk, Wv, Wo):
    global _NC_CACHE
    if _NC_CACHE is None:
        _NC_CACHE = _build_nc()
    nc = _NC_CACHE

    hs = np.asarray(hidden_states, dtype=np.float32)
    B = hs.shape[0]
    assert hs.shape == (B, S, D)
    hst = round_fp32r(np.ascontiguousarray(hs[0].T))  # [D, S]
    cosT, sinT, mask_bf, rt, ident, ones = _host_tables()

    Wq = np.asarray(Wq, dtype=np.float32)
    Wk = np.asarray(Wk, dtype=np.float32)
    Wv = np.asarray(Wv, dtype=np.float32)
    Wo = np.asarray(Wo, dtype=np.float32)

    in_maps = []
    for c in range(NCORES):
        in_maps.append({
            "hst": hst,
            "wq": round_fp32r(Wq[:, c * QH * HD:(c + 1) * QH * HD]),
            "wk": round_fp32r(Wk[:, c * HD:(c + 1) * HD]),
            "wv": round_fp32r(Wv[:, c * HD:(c + 1) * HD]),
            "wo": round_fp32r(Wo[c * QH * HD:(c + 1) * QH * HD, :]),
            "cost": cosT,
            "sint": sinT,
            "maskt": mask_bf,
            "rt": rt,
            "ident": ident,
            "ones": ones,
        })

    import os
    trace = os.environ.get("KERNEL_TRACE") == "1"
    if trace:
        try:
            import antenv.axon_hooks  # noqa: F401  (profiling hook, optional)
        except ImportError:
            trace = False
    res = run_bass_kernel_spmd(nc, in_maps, list(range(NCORES)), trace=trace)
    if trace:
        kernel.last_results = res

    acc = np.zeros((S, D), dtype=np.float64)
    for c in range(NCORES):
        acc += res.results[c]["out"].astype(np.float64)
    return acc.astype(np.float32).reshape(B, S, D)

